# revision 22
# baseline (speedup 1.0000x reference)
"""Trainium2 Bass kernel for nn_DeepSeekWithEmbedding (2-layer LLaMA-style
transformer, H=4096, NH=32, F=11008, B=2, S=512) on 8 NeuronCores.

Sharding: tensor-parallel over 8 cores. Wq/Wk/Wv sharded on output dim
(4 heads per core), Wg/Wu on output dim (1376 F-cols per core, padded to
1408), Wo/Wd/W_out sharded on OUTPUT dim too — instead of all-reducing
16 MB partial sums, the sharded activations (hidden, attention out, MLP
act) are all-gathered and every core computes only its own 512 hidden
columns. RMSNorm statistics are all-reduced as a tiny [1,1024] vector.

Every AllGather is split into [128, 1024]-row chunk-AGs issued as soon as
the producing chunk is ready; consumer contraction loops run chunk-outer /
rank-inner so compute overlaps the gather pipeline.

All activations live in transposed layout [features, tokens] so the PE
contraction dim (partitions) is the feature dim. Large matmuls run in
bfloat16 (1 cycle/row on the PE, 2x the float32r rate); PSUM accumulation,
norm/softmax statistics and the residual stream stay fp32. Attention is
normalized AFTER the AV matmul (o = (exp @ v) * 1/denom) so the causal
mask falls out of the PSUM accumulation ranges and the per-head partition
broadcast of 1/denom is a K=1 PE matmul instead of a DRAM DMA bounce.
"""
import numpy as np
import ml_dtypes

import concourse.bass as bass
import concourse.mybir as mybir
import concourse.tile as tile
from concourse.bass_utils import run_bass_kernel_spmd

f32 = mybir.dt.float32
f32r = mybir.dt.float32r
bf16 = mybir.dt.bfloat16
AF = mybir.ActivationFunctionType

NC = 8
L, H, NH, HD, F = 2, 4096, 32, 128, 11008
B, S, NIN, DOUT = 2, 512, 512, 8
T = B * S                      # 1024 tokens
KC = H // 128                  # 32 k-chunks over H
MH = H // NC // 128            # 4 local head/col chunks per core
FSH = 1408                     # F shard padded (1376 -> 11*128)
FM = FSH // 128                # 11 f-chunks per core
EPS = 1e-6
SCALE = 1.0 / float(np.sqrt(HD))
ROPE_BASE = 10000.0


def _bcast_rows(nc, dst, src, scratch):
    """Broadcast src [1, N] (SBUF) to dst [128, N] via a DRAM bounce: DMA the
    row out, then DMA it back with a stride-0 partition read (full fp32)."""
    n = src.shape[-1]
    nc.sync.dma_start(scratch[:, :n], src[:])
    nc.sync.dma_start(dst[:], scratch[:, :n].to_broadcast([128, n]))


def _fix_inst_waits(nc):
    """HW instruction structs carry at most 1 sync-wait (fp32/f32r matmuls,
    DVE/ACT ops, drains, collectives). Hoist extras onto same-engine NoOps."""
    for fn in nc.m.functions:
        for bb in fn.blocks:
            out = []
            for ins in bb.instructions:
                si = ins.sync_info
                if si is not None and len(si.on_wait) > 1:
                    waits = list(si.on_wait)
                    for w in waits[:-1]:
                        nop = mybir.InstNoOp(
                            name=f"WNOP-{nc.next_id()}", ins=[], outs=[])
                        nop.engine = ins.engine
                        nop.sync_info = mybir.SyncInfo(on_wait=[w], on_update=[])
                        out.append(nop)
                    ins.sync_info = mybir.SyncInfo(
                        on_wait=[waits[-1]], on_update=list(si.on_update))
                out.append(ins)
            bb.instructions[:] = out


def build_bass():
    nc = bass.Bass()
    RG = [list(range(NC))]

    xT_in = nc.dram_tensor("xT", [NIN // 128, 128, T], f32r, kind="ExternalInput")
    wemb_in = nc.dram_tensor("wemb", [128, NIN // 128, 512], f32r, kind="ExternalInput")
    bemb_in = nc.dram_tensor("bemb", [128, MH], f32, kind="ExternalInput")
    wq_in = nc.dram_tensor("wq", [L, 128, KC, 512], bf16, kind="ExternalInput")
    wk_in = nc.dram_tensor("wk", [L, 128, KC, 512], bf16, kind="ExternalInput")
    wv_in = nc.dram_tensor("wv", [L, 128, KC, 512], bf16, kind="ExternalInput")
    wo_in = nc.dram_tensor("wo", [L, 128, KC, 512], bf16, kind="ExternalInput")
    wg_in = nc.dram_tensor("wg", [L, 128, FM, KC, 128], bf16, kind="ExternalInput")
    wu_in = nc.dram_tensor("wu", [L, 128, FM, KC, 128], bf16, kind="ExternalInput")
    wd_in = nc.dram_tensor("wd", [L, 128, NC * FM, 512], bf16, kind="ExternalInput")
    wout_in = nc.dram_tensor("wout", [128, MH, DOUT], f32r, kind="ExternalInput")
    bout_in = nc.dram_tensor("bout", [DOUT, 1], f32, kind="ExternalInput")
    cos_in = nc.dram_tensor("cosT", [128, T], f32, kind="ExternalInput")
    sinx_in = nc.dram_tensor("sinX", [128, T], f32, kind="ExternalInput")
    tri_in = nc.dram_tensor("tri", [128, 128], bf16, kind="ExternalInput")
    ones_in = nc.dram_tensor("onesc", [128, 1], f32r, kind="ExternalInput")
    eps_in = nc.dram_tensor("epsc", [1, 1], f32, kind="ExternalInput")

    out_t = nc.dram_tensor("outT", [DOUT, T], f32, kind="ExternalOutput")

    with tile.TileContext(nc, num_cores=NC) as tc:
        with (
            tc.tile_pool(name="persist", bufs=1) as pp,
            tc.tile_pool(name="dram", bufs=2, space="DRAM") as dp,
        ):
            cosT = pp.tile([128, T], f32)
            sinX = pp.tile([128, T], f32)
            cosR = pp.tile([128, T], f32)
            sinR = pp.tile([128, T], f32)
            rbc_t = pp.tile([128, T], f32)
            tri = pp.tile([128, 128], bf16)
            ones = pp.tile([128, 1], f32r)
            ones_bf = pp.tile([128, 1], bf16)
            ones_row = pp.tile([1, 128], bf16)
            eps_t = pp.tile([1, 1], f32)
            bemb = pp.tile([128, MH], f32)
            bout = pp.tile([DOUT, 1], f32)
            h_resT = pp.tile([128, MH, T], f32)      # residual shard (own cols)

            nc.sync.dma_start(cosT[:], cos_in[:])
            nc.sync.dma_start(sinX[:], sinx_in[:])
            nc.sync.dma_start(tri[:], tri_in[:])
            nc.sync.dma_start(ones[:], ones_in[:])
            nc.vector.memset(ones_bf[:], 1.0)
            nc.vector.memset(ones_row[:], 1.0)
            nc.sync.dma_start(eps_t[:], eps_in[:])
            nc.sync.dma_start(bemb[:], bemb_in[:])
            nc.sync.dma_start(bout[:], bout_in[:])

            # ================= embedding: h = x @ W_emb + b_emb =========
            with (
                tc.tile_pool(name="emb", bufs=2) as ep,
                tc.tile_pool(name="emb_ps", bufs=1, space="PSUM") as eps_ps,
            ):
                xT_sb = ep.tile([128, NIN // 128, T], f32r, name="xT_sb")
                wemb_sb = ep.tile([128, NIN // 128, 512], f32r, name="wemb_sb")
                for k in range(NIN // 128):
                    nc.sync.dma_start(xT_sb[:, k, :], xT_in[k])
                    nc.sync.dma_start(wemb_sb[:, k, :], wemb_in[:, k, :])
                for m in range(MH):
                    for hf in range(2):
                        ps = eps_ps.tile([128, 512], f32, name=f"eps_{m}_{hf}",
                                         tag=f"embps{m}{hf}")
                        for k in range(NIN // 128):
                            nc.tensor.matmul(
                                ps[:], wemb_sb[:, k, bass.ts(m, 128)],
                                xT_sb[:, k, bass.ts(hf, 512)],
                                start=(k == 0), stop=(k == NIN // 128 - 1))
                        nc.vector.tensor_scalar_add(
                            h_resT[:, m, bass.ts(hf, 512)], ps[:],
                            bemb[:, m:m + 1])

            def norm_gather(tag):
                """AG the UNNORMALIZED residual shard in bf16 (2 chunks of 2
                m-blocks, concurrent with the sumsq AllReduce); the 1/rms
                factor lands in rbc_t and is folded into downstream consumers.
                gouts[j][256*r + 128*kk, :] holds H-chunk k = 4*r + 2*j + kk."""
                ar_in = dp.tile([1, T], f32, name=f"arin_{tag}", tag="ar_ss_in")
                ar_out = dp.tile([1, T], f32, name=f"arout_{tag}",
                                 tag="ar_ss_out", addr_space="Shared")
                gouts = []
                with (
                    tc.tile_pool(name=f"nrm_{tag}", bufs=2) as np_,
                    tc.tile_pool(name=f"nrm_ps_{tag}", bufs=1, space="PSUM") as ps_,
                ):
                    for j in range(2):
                        gin = dp.tile([256, T], bf16, name=f"ghi_{tag}{j}",
                                      tag=f"aghi{j}")
                        gout = dp.tile([NC * 256, T], bf16, name=f"gho_{tag}{j}",
                                       tag=f"agho{j}", addr_space="Shared")
                        for kk in range(2):
                            hb = np_.tile([128, T], bf16, name=f"hb_{tag}",
                                          tag="hb")
                            nc.scalar.activation(hb[:], h_resT[:, 2 * j + kk, :],
                                                 AF.Copy)
                            nc.sync.dma_start(gin[bass.ts(kk, 128), :], hb[:])
                        nc.gpsimd.collective_compute(
                            "AllGather", mybir.AluOpType.bypass,
                            replica_groups=RG,
                            ins=[gin[:].opt()], outs=[gout[:].opt()])
                        gouts.append(gout)
                    ssp = ps_.tile([1, T], f32, name=f"ssp_{tag}", tag="ssp")
                    for m in range(MH):
                        sq = np_.tile([128, T], f32r, name=f"sq_{tag}", tag="sq")
                        nc.scalar.activation(sq[:], h_resT[:, m, :], AF.Square)
                        for hf in range(2):
                            nc.tensor.matmul(
                                ssp[:, bass.ts(hf, 512)], ones[:],
                                sq[:, bass.ts(hf, 512)],
                                start=(m == 0), stop=(m == MH - 1),
                                skip_group_check=True)
                    ss_sb = np_.tile([1, T], f32, name=f"ss_{tag}", tag="ss_sb")
                    nc.scalar.activation(ss_sb[:], ssp[:], AF.Copy)
                    nc.sync.dma_start(ar_in[:], ss_sb[:])
                    nc.gpsimd.collective_compute(
                        "AllReduce", mybir.AluOpType.add, replica_groups=RG,
                        ins=[ar_in[:].opt()], outs=[ar_out[:].opt()])
                    ssf = np_.tile([1, T], f32, name=f"ssf_{tag}", tag="ssf")
                    nc.sync.dma_start(ssf[:], ar_out[:])
                    srt = np_.tile([1, T], f32, name=f"srt_{tag}", tag="srt")
                    nc.scalar.activation(srt[:], ssf[:], AF.Sqrt,
                                         scale=1.0 / H, bias=eps_t[:])
                    rsq = np_.tile([1, T], f32, name=f"rsq_{tag}", tag="rsq")
                    nc.vector.reciprocal(rsq[:], srt[:])
                    bsc = dp.tile([1, T], f32, name=f"bsc_{tag}", tag="bc_scr")
                    _bcast_rows(nc, rbc_t, rsq, bsc)
                return gouts

            for l in range(L):
                hn_ag = norm_gather(f"l{l}a")
                nc.vector.tensor_mul(cosR[:], cosT[:], rbc_t[:])
                nc.vector.tensor_mul(sinR[:], sinX[:], rbc_t[:])

                lp = tc.tile_pool(name=f"lat_{l}", bufs=1)
                lpp = lp.__enter__()
                qT = lpp.tile([128, MH, T], bf16, name=f"qT_{l}")
                kT = lpp.tile([128, MH, T], bf16, name=f"kT_{l}")
                v_sb = lpp.tile([128, T // 128, 512], bf16, name=f"v_{l}")
                oT_sb = lpp.tile([128, MH, T], bf16, name=f"oT_{l}")

                # ======= q/k/v projections (chunk-outer, rank-inner) =====
                def proj_qk(w_dram, dstT):
                    with (
                        tc.tile_pool(name="pqk", bufs=4) as sp,
                        tc.tile_pool(name="pqk_ps", bufs=1, space="PSUM") as psp,
                        tc.tile_pool(name="pqk_sc", bufs=2) as scp,
                    ):
                        pss = [psp.tile([128, 512], f32, name=f"pq_{m}_{hf}",
                                        tag=f"pq{m}{hf}")
                               for m in range(MH) for hf in range(2)]
                        first = True
                        for cj in range(2):
                            for r in range(NC):
                                for kk in range(2):
                                    k = 4 * r + 2 * cj + kk
                                    hc = sp.tile([128, T], bf16, name="hc",
                                                 tag="hc")
                                    wc = sp.tile([128, 512], bf16, name="wc",
                                                 tag="wc")
                                    nc.sync.dma_start(
                                        hc[:],
                                        hn_ag[cj][256 * r + 128 * kk:
                                                  256 * r + 128 * (kk + 1), :])
                                    nc.sync.dma_start(wc[:], w_dram[l, :, k, :])
                                    for m in range(MH):
                                        for hf in range(2):
                                            nc.tensor.matmul(
                                                pss[m * 2 + hf][:],
                                                wc[:, bass.ts(m, 128)],
                                                hc[:, bass.ts(hf, 512)],
                                                start=first,
                                                stop=(cj == 1 and r == NC - 1
                                                      and kk == 1))
                                    first = False
                        for m in range(MH):
                            for hf in range(2):
                                ps = pss[m * 2 + hf]
                                t1 = scp.tile([128, 512], f32, name="t1", tag="t1")
                                t2 = scp.tile([128, 512], f32, name="t2", tag="t2")
                                cr = cosR[:, bass.ts(hf, 512)]
                                sr = sinR[:, bass.ts(hf, 512)]
                                nc.vector.tensor_mul(t1[:], ps[:], cr)
                                nc.vector.tensor_mul(t2[0:64, :], ps[64:128, :],
                                                     sr[0:64, :])
                                nc.vector.tensor_mul(t2[64:128, :], ps[0:64, :],
                                                     sr[64:128, :])
                                nc.vector.tensor_add(
                                    dstT[:, m, bass.ts(hf, 512)], t1[:], t2[:])

                proj_qk(wq_in, qT)
                proj_qk(wk_in, kT)

                # v: natural layout [token, vcols]
                with (
                    tc.tile_pool(name="pv", bufs=4) as sp,
                    tc.tile_pool(name="pv_ps", bufs=1, space="PSUM") as psp,
                ):
                    psv = [psp.tile([128, 512], f32, name=f"pv_{t}", tag=f"pv{t}")
                           for t in range(T // 128)]
                    first = True
                    for cj in range(2):
                        for r in range(NC):
                            for kk in range(2):
                                k = 4 * r + 2 * cj + kk
                                hc = sp.tile([128, T], bf16, name="hcv", tag="hcv")
                                hcn = sp.tile([128, T], bf16, name="hcnv",
                                              tag="hcnv")
                                wc = sp.tile([128, 512], bf16, name="wcv",
                                             tag="wcv")
                                nc.sync.dma_start(
                                    hc[:],
                                    hn_ag[cj][256 * r + 128 * kk:
                                              256 * r + 128 * (kk + 1), :])
                                nc.vector.tensor_mul(hcn[:], hc[:], rbc_t[:])
                                nc.sync.dma_start(wc[:], wv_in[l, :, k, :])
                                for t in range(T // 128):
                                    nc.tensor.matmul(
                                        psv[t][:], hcn[:, bass.ts(t, 128)], wc[:],
                                        start=first,
                                        stop=(cj == 1 and r == NC - 1
                                              and kk == 1))
                                first = False
                    for t in range(T // 128):
                        nc.scalar.activation(v_sb[:, t, :], psv[t][:], AF.Copy)

                # ========== attention per head; chunk-AG o per head ======
                o_ag = []
                with (
                    tc.tile_pool(name="att", bufs=2) as ap,
                    tc.tile_pool(name="att_ps", bufs=2, space="PSUM") as aps,
                ):
                    for hd in range(MH):
                        for b in range(B):
                            qh = qT[:, hd, bass.ts(b, 512)]
                            kh = kT[:, hd, bass.ts(b, 512)]
                            expT = ap.tile([128, 4, S], bf16, name="expT",
                                           tag="expT")
                            for kc in range(4):
                                scp = aps.tile([128, S], f32, name="scp",
                                               tag="scp")
                                qlo = kc * 128
                                nc.tensor.matmul(
                                    scp[:, qlo:S], kh[:, bass.ts(kc, 128)],
                                    qh[:, qlo:S], start=True, stop=True)
                                nc.scalar.activation(
                                    expT[:, kc, qlo:S], scp[:, qlo:S], AF.Exp,
                                    scale=SCALE)
                                nc.vector.tensor_mul(
                                    expT[:, kc, qlo:qlo + 128],
                                    expT[:, kc, qlo:qlo + 128], tri[:])
                            dnp = aps.tile([1, S], f32, name="dnp", tag="dnp")
                            for kc in range(4):
                                qlo = kc * 128
                                nc.tensor.matmul(
                                    dnp[:, qlo:S], ones_bf[:],
                                    expT[:, kc, qlo:S],
                                    start=(kc == 0), stop=(kc == 3),
                                    skip_group_check=True)
                            # o = (exp @ v) * 1/denom; causal mask falls out
                            # of the per-kc PSUM column ranges.
                            op = aps.tile([128, S], f32, name="op", tag="op")
                            for kc in range(4):
                                qlo = kc * 128
                                nc.tensor.matmul(
                                    op[:, qlo:S],
                                    v_sb[:, b * 4 + kc, bass.ts(hd, 128)],
                                    expT[:, kc, qlo:S], start=(kc == 0),
                                    stop=(kc == 3), skip_group_check=True)
                            rcp = ap.tile([1, S], bf16, name="rcp", tag="rcp")
                            with nc.allow_low_precision(
                                    reason="softmax 1/denom in bf16"):
                                nc.vector.reciprocal(rcp[:], dnp[:])
                            rbc_ps = aps.tile([128, S], f32, name="rbcp",
                                              tag="rbcp")
                            nc.tensor.matmul(rbc_ps[:], ones_row[:], rcp[:],
                                             start=True, stop=True)
                            nc.vector.tensor_mul(oT_sb[:, hd, bass.ts(b, 512)],
                                                 op[:], rbc_ps[:])
                        if hd % 2 == 1:
                            j = hd // 2
                            gin = dp.tile([256, T], bf16, name=f"goi_{l}{j}",
                                          tag=f"agoi{j}")
                            gout = dp.tile([NC * 256, T], bf16,
                                           name=f"goo_{l}{j}", tag=f"agoo{j}",
                                           addr_space="Shared")
                            nc.sync.dma_start(gin[0:128, :],
                                              oT_sb[:, hd - 1, :])
                            nc.sync.dma_start(gin[128:256, :], oT_sb[:, hd, :])
                            nc.gpsimd.collective_compute(
                                "AllGather", mybir.AluOpType.bypass,
                                replica_groups=RG,
                                ins=[gin[:].opt()], outs=[gout[:].opt()])
                            o_ag.append(gout)
                lp.__exit__(None, None, None)

                # ======= attn out-proj: h += oT_full-contract @ Wo =======
                with (
                    tc.tile_pool(name="pwo", bufs=4) as sp,
                    tc.tile_pool(name="pwo_ps", bufs=1, space="PSUM") as psp,
                ):
                    pso = [psp.tile([128, 512], f32, name=f"po_{m}_{hf}",
                                    tag=f"po{m}{hf}")
                           for m in range(MH) for hf in range(2)]
                    first = True
                    for cj in range(2):
                        for r in range(NC):
                            for kk in range(2):
                                k = 4 * r + 2 * cj + kk
                                oc = sp.tile([128, T], bf16, name="oc", tag="oc")
                                wc = sp.tile([128, 512], bf16, name="woc",
                                             tag="woc")
                                nc.sync.dma_start(
                                    oc[:],
                                    o_ag[cj][256 * r + 128 * kk:
                                             256 * r + 128 * (kk + 1), :])
                                nc.sync.dma_start(wc[:], wo_in[l, :, k, :])
                                for m in range(MH):
                                    for hf in range(2):
                                        nc.tensor.matmul(
                                            pso[m * 2 + hf][:],
                                            wc[:, bass.ts(m, 128)],
                                            oc[:, bass.ts(hf, 512)],
                                            start=first,
                                            stop=(cj == 1 and r == NC - 1
                                                  and kk == 1))
                                first = False
                    for m in range(MH):
                        for hf in range(2):
                            nc.vector.tensor_add(
                                h_resT[:, m, bass.ts(hf, 512)],
                                h_resT[:, m, bass.ts(hf, 512)],
                                pso[m * 2 + hf][:])

                hn_ag2 = norm_gather(f"l{l}b")

                # ======= MLP gate/up (hnT cached, fm-outer); chunk-AG a ==
                a_ag = []
                with (
                    tc.tile_pool(name="mlp_hn", bufs=1) as hp,
                    tc.tile_pool(name="mlp", bufs=2) as sp,
                    tc.tile_pool(name="mlp_ps", bufs=2, space="PSUM") as psp,
                ):
                    hnc = hp.tile([128, KC, T], bf16, name="hnc")
                    for cj in range(2):
                        for r in range(NC):
                            for kk in range(2):
                                k = 4 * r + 2 * cj + kk
                                hcr = sp.tile([128, T], bf16, name="hcr",
                                              tag="hcr", bufs=2)
                                nc.sync.dma_start(
                                    hcr[:],
                                    hn_ag2[cj][256 * r + 128 * kk:
                                               256 * r + 128 * (kk + 1), :])
                                nc.vector.tensor_mul(hnc[:, k, :], hcr[:],
                                                     rbc_t[:])
                    for fm in range(FM):
                        pg = psp.tile([128, T], f32, name="pg", tag="pg")
                        pu = psp.tile([128, T], f32, name="pu", tag="pu")
                        for kg in range(4):
                            wgt = sp.tile([128, 8, 128], bf16, name="wgt",
                                          tag="wgt", bufs=2)
                            wut = sp.tile([128, 8, 128], bf16, name="wut",
                                          tag="wut", bufs=2)
                            nc.sync.dma_start(wgt[:],
                                              wg_in[l, :, fm, bass.ts(kg, 8), :])
                            nc.sync.dma_start(wut[:],
                                              wu_in[l, :, fm, bass.ts(kg, 8), :])
                            for kk in range(8):
                                k = kg * 8 + kk
                                for hf in range(2):
                                    nc.tensor.matmul(
                                        pg[:, bass.ts(hf, 512)], wgt[:, kk, :],
                                        hnc[:, k, bass.ts(hf, 512)],
                                        start=(k == 0), stop=(k == KC - 1))
                                    nc.tensor.matmul(
                                        pu[:, bass.ts(hf, 512)], wut[:, kk, :],
                                        hnc[:, k, bass.ts(hf, 512)],
                                        start=(k == 0), stop=(k == KC - 1))
                        sg = sp.tile([128, T], f32, name="sg", tag="sg", bufs=1)
                        nc.scalar.activation(sg[:], pg[:], AF.Silu)
                        ab = sp.tile([128, T], bf16, name="ab", tag="ab", bufs=2)
                        nc.vector.tensor_mul(ab[:], sg[:], pu[:])
                        j, sub = divmod(fm, 2)
                        if fm == FM - 1:            # last odd chunk: 1 block
                            gin = dp.tile([128, T], bf16, name=f"gai_{l}{j}",
                                          tag=f"agai{j}")
                            gout = dp.tile([NC * 128, T], bf16,
                                           name=f"gao_{l}{j}", tag=f"agao{j}",
                                           addr_space="Shared")
                            nc.sync.dma_start(gin[:], ab[:])
                            nc.gpsimd.collective_compute(
                                "AllGather", mybir.AluOpType.bypass,
                                replica_groups=RG,
                                ins=[gin[:].opt()], outs=[gout[:].opt()])
                            a_ag.append(gout)
                        else:
                            if sub == 0:
                                gin = dp.tile([256, T], bf16, name=f"gai_{l}{j}",
                                              tag=f"agai{j}")
                                a_gin = gin
                            else:
                                gin = a_gin
                            nc.sync.dma_start(
                                gin[bass.ts(sub, 128), :], ab[:])
                            if sub == 1:
                                gout = dp.tile([NC * 256, T], bf16,
                                               name=f"gao_{l}{j}",
                                               tag=f"agao{j}",
                                               addr_space="Shared")
                                nc.gpsimd.collective_compute(
                                    "AllGather", mybir.AluOpType.bypass,
                                    replica_groups=RG,
                                    ins=[gin[:].opt()], outs=[gout[:].opt()])
                                a_ag.append(gout)

                # ======= MLP down: h += aT_full-contract @ Wd ============
                with (
                    tc.tile_pool(name="pwd", bufs=4) as sp,
                    tc.tile_pool(name="pwd_ps", bufs=1, space="PSUM") as psp,
                ):
                    psd = [psp.tile([128, 512], f32, name=f"pd_{m}_{hf}",
                                    tag=f"pd{m}{hf}")
                           for m in range(MH) for hf in range(2)]
                    first = True
                    for j in range(6):
                        nsub = 1 if j == 5 else 2
                        for r in range(NC):
                            for sub in range(nsub):
                                fm = 2 * j + sub
                                kc = r * FM + fm
                                ac = sp.tile([128, T], bf16, name="ac", tag="ac")
                                wc = sp.tile([128, 512], bf16, name="wdc",
                                             tag="wdc")
                                nc.sync.dma_start(
                                    ac[:],
                                    a_ag[j][nsub * 128 * r + 128 * sub:
                                            nsub * 128 * r + 128 * (sub + 1), :])
                                nc.sync.dma_start(wc[:], wd_in[l, :, kc, :])
                                for m in range(MH):
                                    for hf in range(2):
                                        nc.tensor.matmul(
                                            psd[m * 2 + hf][:],
                                            wc[:, bass.ts(m, 128)],
                                            ac[:, bass.ts(hf, 512)],
                                            start=first,
                                            stop=(j == 5 and r == NC - 1))
                                first = False
                    for m in range(MH):
                        for hf in range(2):
                            nc.vector.tensor_add(
                                h_resT[:, m, bass.ts(hf, 512)],
                                h_resT[:, m, bass.ts(hf, 512)],
                                psd[m * 2 + hf][:])

            # ================= final norm + out head ====================
            ar2_in = dp.tile([1, T], f32, name="arin_f", tag="ar_ss_in")
            ar2_out = dp.tile([1, T], f32, name="arout_f", tag="ar_ss_out",
                              addr_space="Shared")
            aro_in = dp.tile([DOUT, T], f32, name="aroin")
            aro_out = dp.tile([DOUT, T], f32, name="aroout", addr_space="Shared")
            with (
                tc.tile_pool(name="fin", bufs=2) as np_,
                tc.tile_pool(name="fin_ps", bufs=1, space="PSUM") as ps_,
            ):
                ssp = ps_.tile([1, T], f32, name="ssp_f", tag="sspf")
                for m in range(MH):
                    sq = np_.tile([128, T], f32r, name="sq_f", tag="sqf")
                    nc.scalar.activation(sq[:], h_resT[:, m, :], AF.Square)
                    for hf in range(2):
                        nc.tensor.matmul(
                            ssp[:, bass.ts(hf, 512)], ones[:],
                            sq[:, bass.ts(hf, 512)],
                            start=(m == 0), stop=(m == MH - 1),
                            skip_group_check=True)
                ss_sb = np_.tile([1, T], f32, name="ss_f")
                nc.scalar.activation(ss_sb[:], ssp[:], AF.Copy)
                nc.sync.dma_start(ar2_in[:], ss_sb[:])
                nc.gpsimd.collective_compute(
                    "AllReduce", mybir.AluOpType.add, replica_groups=RG,
                    ins=[ar2_in[:].opt()], outs=[ar2_out[:].opt()])
                ssf = np_.tile([1, T], f32, name="ssf_f")
                nc.sync.dma_start(ssf[:], ar2_out[:])
                srt = np_.tile([1, T], f32, name="srt_f")
                nc.scalar.activation(srt[:], ssf[:], AF.Sqrt, scale=1.0 / H,
                                     bias=eps_t[:])
                rsq = np_.tile([1, T], f32, name="rsq_f")
                nc.vector.reciprocal(rsq[:], srt[:])
                rbc = np_.tile([128, T], f32, name="rbc_f")
                bsc_f = dp.tile([1, T], f32, name="bsc_f", tag="bc_scr")
                _bcast_rows(nc, rbc, rsq, bsc_f)
                hn_f = np_.tile([128, MH, T], f32r, name="hn_f")
                for m in range(MH):
                    nc.vector.tensor_mul(hn_f[:, m, :], h_resT[:, m, :], rbc[:])
                wout_sb = np_.tile([128, MH, DOUT], f32r, name="wout_sb")
                nc.sync.dma_start(wout_sb[:], wout_in[:])
                pout = ps_.tile([DOUT, T], f32, name="pout")
                for m in range(MH):
                    for hf in range(2):
                        nc.tensor.matmul(
                            pout[:, bass.ts(hf, 512)], wout_sb[:, m, :],
                            hn_f[:, m, bass.ts(hf, 512)],
                            start=(m == 0), stop=(m == MH - 1),
                            skip_group_check=True)
                po_sb = np_.tile([DOUT, T], f32, name="po_sb")
                nc.scalar.activation(po_sb[:], pout[:], AF.Copy)
                nc.sync.dma_start(aro_in[:], po_sb[:])
                nc.gpsimd.collective_compute(
                    "AllReduce", mybir.AluOpType.add, replica_groups=RG,
                    ins=[aro_in[:].opt()], outs=[aro_out[:].opt()])
                fo = np_.tile([DOUT, T], f32, name="fo")
                nc.sync.dma_start(fo[:], aro_out[:])
                nc.vector.tensor_scalar_add(fo[:], fo[:], bout[:])
                nc.sync.dma_start(out_t[:], fo[:])

    _fix_inst_waits(nc)
    return nc


def prepare_inputs(inputs):
    """Host-side sharding + layout. Returns in_maps for the 8 cores."""
    x = np.asarray(inputs["x"], np.float32)
    W_emb = np.asarray(inputs["W_emb"], np.float32)
    b_emb = np.asarray(inputs["b_emb"], np.float32)
    ln1 = np.asarray(inputs["ln1_w"], np.float32)
    Wq = np.asarray(inputs["Wq"], np.float32)
    Wk = np.asarray(inputs["Wk"], np.float32)
    Wv = np.asarray(inputs["Wv"], np.float32)
    Wo = np.asarray(inputs["Wo"], np.float32)
    ln2 = np.asarray(inputs["ln2_w"], np.float32)
    Wg = np.asarray(inputs["Wg"], np.float32)
    Wu = np.asarray(inputs["Wu"], np.float32)
    Wd = np.asarray(inputs["Wd"], np.float32)
    lnf = np.asarray(inputs["lnf_w"], np.float32)
    W_out = np.asarray(inputs["W_out"], np.float32)
    b_out = np.asarray(inputs["b_out"], np.float32)

    # fold rmsnorm gains into following projections (exact for unit gains)
    Wq_f = ln1[:, :, None] * Wq
    Wk_f = ln1[:, :, None] * Wk
    Wv_f = ln1[:, :, None] * Wv
    Wg_f = ln2[:, :, None] * Wg
    Wu_f = ln2[:, :, None] * Wu
    Wout_f = lnf[:, None] * W_out

    xT = np.ascontiguousarray(
        x.reshape(T, NIN).T.reshape(NIN // 128, 128, T))

    pos = np.arange(S, dtype=np.float64)
    inv_freq = 1.0 / (ROPE_BASE ** (np.arange(0, HD, 2) / HD))
    emb = np.concatenate([pos[:, None] * inv_freq[None, :]] * 2, axis=1)
    cosT = np.tile(np.cos(emb).T.astype(np.float32), (1, B))
    sinT = np.tile(np.sin(emb).T.astype(np.float32), (1, B))
    sinX = sinT.copy()
    sinX[:64] *= -1.0
    bf = ml_dtypes.bfloat16
    tri = np.ascontiguousarray(np.triu(np.ones((128, 128), bf)))
    onesc = np.ones((128, 1), np.float32)
    epsc = np.full((1, 1), EPS, np.float32)

    def colshard(w, c):           # [L, H, Hcols] -> [L, 128, KC, 512]
        s = w[:, :, 512 * c:512 * (c + 1)]
        return np.ascontiguousarray(
            s.reshape(L, KC, 128, 512).transpose(0, 2, 1, 3)).astype(bf)

    in_maps = []
    for c in range(NC):
        wg_c = np.zeros((L, H, FSH), np.float32)
        wg_c[:, :, :1376] = Wg_f[:, :, 1376 * c:1376 * (c + 1)]
        wu_c = np.zeros((L, H, FSH), np.float32)
        wu_c[:, :, :1376] = Wu_f[:, :, 1376 * c:1376 * (c + 1)]
        wg_l = np.ascontiguousarray(
            wg_c.reshape(L, KC, 128, FM, 128).transpose(0, 2, 3, 1, 4)).astype(bf)
        wu_l = np.ascontiguousarray(
            wu_c.reshape(L, KC, 128, FM, 128).transpose(0, 2, 3, 1, 4)).astype(bf)
        # Wd rows in rank-padded order: [L, NC*FSH, 512cols]
        wd_c = np.zeros((L, NC * FSH, 512), np.float32)
        for r in range(NC):
            wd_c[:, r * FSH:r * FSH + 1376] = \
                Wd[:, 1376 * r:1376 * (r + 1), 512 * c:512 * (c + 1)]
        wd_l = np.ascontiguousarray(
            wd_c.reshape(L, NC * FM, 128, 512).transpose(0, 2, 1, 3)).astype(bf)
        wemb_c = np.ascontiguousarray(
            W_emb[:, 512 * c:512 * (c + 1)]
            .reshape(NIN // 128, 128, 512).transpose(1, 0, 2))
        bemb_c = np.ascontiguousarray(
            b_emb[512 * c:512 * (c + 1)].reshape(MH, 128).T)
        wout_c = np.ascontiguousarray(
            Wout_f[512 * c:512 * (c + 1)].reshape(MH, 128, DOUT)
            .transpose(1, 0, 2))
        in_maps.append({
            "xT": xT, "wemb": wemb_c, "bemb": bemb_c,
            "wq": colshard(Wq_f, c), "wk": colshard(Wk_f, c),
            "wv": colshard(Wv_f, c), "wo": colshard(Wo, c),
            "wg": wg_l, "wu": wu_l, "wd": wd_l,
            "wout": wout_c, "bout": b_out.reshape(DOUT, 1),
            "cosT": cosT, "sinX": sinX, "tri": tri,
            "onesc": onesc, "epsc": epsc,
        })
    return in_maps


_NC_CACHE = {}


def get_nc():
    if "nc" not in _NC_CACHE:
        _NC_CACHE["nc"] = build_bass()
    return _NC_CACHE["nc"]


def kernel(**inputs):
    nc = get_nc()
    in_maps = prepare_inputs(inputs)
    res = run_bass_kernel_spmd(nc, in_maps, list(range(NC)))
    outT = res.results[0]["outT"]
    return np.ascontiguousarray(outT.T.reshape(B, S, DOUT)).astype(np.float32)

